# revision 25
# baseline (speedup 1.0000x reference)
"""Trainium2 Bass kernel for nn_MessagePassing (gnn_message_passing).

Self-contained: takes full (unsharded) numpy inputs, shards batch*rounds
across 8 NeuronCores, runs a Bass/Tile kernel per core, gathers the full
output.

Math (per (b,r) group, all biases included):
  q      = Wq @ ques + bq                       [H]
  edges  = W1a @ on + W1b @ adj + b1            [H, N*E]  (on broadcast over E)
  a      = softmax_E(We @ (q*edges) + be)       -> folded:  (We*diag(q)) @ edges
  edges2 = a * edges
  t      = W2a @ adj + W2b @ edges2 + b2
  b      = softmax_E(Wv @ (q*t) + bv)           -> folded:  (Wv*diag(q)) @ t
  out    = sum_E b * (Wadj @ adj + badj)        [H, N]

Precision scheme: the softmax-logit paths (stages A/B/E/F) run as fp8
DoubleRow matmuls (2 contraction rows per PE cell, ~1.8x streaming rate).
fp8 noise there is damped by the near-uniform softmaxes (logit std ~0.1),
so it barely reaches the output. The direct value path (Wadj @ adj, stage
H) stays bf16. fp8 weights are scaled x32 so w*32 ~ N(0, 0.64) clears the
e4m3 subnormal floor; intermediates (edges, t, edges2) are stored x32 in
fp8 and the exp() activations unscale by 1/1024 (= 1/32 * 1/32).

Layout on device: hidden channels on partitions (4 chunks of 128), tokens
(node*E+e) on the free dim, so softmax over E is a free-dim segment reduce.
PSUM is used as 2-bank [128, 2, 512] tiles so each stage drain is one
strided DVE/ACT instruction over 800 tokens instead of two 400-token ones.

Schedule: groups are software-pipelined — the front half (loads, q-fold,
edges, softmax-a chain) of group g is emitted before the back half
(t/softmax-b/output) of group g-1, so the PE always has a ready matmul
stage while the DVE/ACT/GPSIMD softmax chain of the newer group runs.
"""

import os
import sys

for _p in ("/opt/trn_rl_repo", "/root/.axon_site/_ro/trn_rl_repo",
           "/root/.axon_site/_ro/pypackages"):
    if _p not in sys.path and os.path.isdir(_p):
        sys.path.append(_p)

import contextlib
import ctypes
import types

import ml_dtypes
import numpy as np

import concourse.bass as bass
import concourse.tile as tile
from concourse import mybir

BF = mybir.dt.bfloat16
F32 = mybir.dt.float32
F16 = mybir.dt.float16
F8 = mybir.dt.float8e4
AX = mybir.AxisListType
ALU = mybir.AluOpType
ACTF = mybir.ActivationFunctionType
DR = mybir.MatmulPerfMode.DoubleRow

B, R, N, E, D, H = 4, 10, 80, 20, 300, 512
BR = B * R              # 40 (b,r) groups
NCORES = 8
G = BR // NCORES        # 5 groups per core
TOK = N * E             # 1600 tokens per group
NT = 4                  # token tiles per group
T = TOK // NT           # 400 tokens per tile
TN = N // NT            # 20 nodes per tile

KD = [(0, 128), (128, 256), (256, 300)]               # D=300 contraction chunks
KH = [(0, 128), (128, 256), (256, 384), (384, 512)]   # H=512 contraction chunks
MS = [(0, 128), (128, 256), (256, 384), (384, 512)]   # output chunks

SC = 32.0               # fp8 weight/intermediate scale
ISC2 = 1.0 / (SC * SC)  # unscale for exp() after (32w) @ (32x)

_MAXW = 1  # this walrus build allows a single semaphore wait per instruction


def _split_multi_waits(nc):
    """Walrus here rejects instructions with >1 sem wait; hoist extra waits
    onto same-engine NoOps inserted just before the instruction."""
    ctr = 0
    for fn in nc.m.functions:
        for bb in fn.blocks:
            new = []
            for inst in bb.instructions:
                si = inst.sync_info
                if si is not None:
                    waits = list(si.on_wait)
                    if len(waits) > _MAXW:
                        for i in range(0, len(waits) - _MAXW, _MAXW):
                            ctr += 1
                            nop = mybir.InstNoOp(name=f"wsplit-{ctr}")
                            nop.engine = inst.engine
                            nop.sync_info = mybir.SyncInfo(
                                on_wait=waits[i : i + _MAXW], on_update=[]
                            )
                            new.append(nop)
                        si.on_wait = waits[len(waits) - _MAXW :]
                new.append(inst)
            bb.instructions = new
    return ctr


def _patch_ldw_dedupe():
    """The bass pipeline splits every matmul into Ldweights + Matmult.
    Consecutive matmuls that share the stationary operand then reload the
    same weights. Drop the redundant Ldweights at the BIR-JSON level
    (walrus's own --enable-ldw-opt rejects explicit Ldweights)."""
    import orjson

    import concourse.bass2jax as b2j
    import concourse.bass_utils as bu

    if getattr(bu, "_ldw_dedupe_patched", False):
        return
    orig = bu.compile_bir_kernel

    def _dedupe(bir_json):
        d = orjson.loads(bir_json)
        removed = 0
        nopctr = 0
        for fn in d.get("functions", []):
            stack = list(fn.get("blocks", []))
            while stack:
                blk = stack.pop()
                stack.extend(blk.get("blocks", []))
                insts = blk.get("instructions", [])
                out = []
                last_key = None
                for i in insts:
                    op = i.get("opcode")
                    if op == "Ldweights":
                        key = orjson.dumps(
                            [
                                i.get("ins"),
                                i.get("perf_mode"),
                                i.get("tile_position"),
                                i.get("tile_size"),
                                i.get("is_transpose"),
                            ]
                        )
                        si = i.get("sync_info") or {}
                        if key == last_key and not si.get("on_update"):
                            w = si.get("on_wait") or []
                            if w:
                                nopctr += 1
                                out.append(
                                    {
                                        "name": f"ldwkeep-{nopctr}",
                                        "opcode": "NoOp",
                                        "engine": i.get("engine", "PE"),
                                        "ins": [],
                                        "outs": [],
                                        "sync_info": {
                                            "on_wait": w,
                                            "on_update": [],
                                        },
                                    }
                                )
                            removed += 1
                            continue
                        last_key = key
                    elif op == "Matmult":
                        if i.get("is_transpose") or i.get("ldweights"):
                            last_key = None
                    out.append(i)
                blk["instructions"] = out
        if os.environ.get("KERNEL_DEBUG"):
            print(f"ldw dedupe: removed {removed}", file=sys.stderr)
        return orjson.dumps(d)

    def compile_bir_kernel(bir_json, tmpdir, neff_name="file.neff"):
        try:
            bir_json = _dedupe(bir_json)
        except Exception as e:  # pragma: no cover - safety net
            print(f"ldw dedupe skipped: {e}", file=sys.stderr)
        return orig(bir_json, tmpdir, neff_name=neff_name)

    bu.compile_bir_kernel = compile_bir_kernel
    b2j.compile_bir_kernel = compile_bir_kernel
    bu._ldw_dedupe_patched = True


def _install_ntff_hook():
    """Provide antenv.axon_hooks (missing in this image) so that
    run_bass_kernel_spmd(trace=True) can profile via libaxon_pjrt."""
    if "antenv.axon_hooks" in sys.modules:
        return

    def _mk(so_path):
        try:
            lib = ctypes.CDLL(so_path)
        except OSError:
            return None
        if not hasattr(lib, "axon_start_nrt_profile"):
            return None
        lib.axon_start_nrt_profile.argtypes = [
            ctypes.POINTER(ctypes.c_int64),
            ctypes.c_size_t,
        ]
        lib.axon_start_nrt_profile.restype = ctypes.c_int64
        lib.axon_stop_nrt_profile.argtypes = [ctypes.c_char_p]
        lib.axon_stop_nrt_profile.restype = ctypes.c_int64

        @contextlib.contextmanager
        def _hook(output_dir, device_ids):
            import jax

            jax.devices()
            if device_ids:
                ids = (ctypes.c_int64 * len(device_ids))(*device_ids)
                rc = lib.axon_start_nrt_profile(ids, len(device_ids))
            else:
                rc = lib.axon_start_nrt_profile(None, 0)
            if rc != 0:
                raise RuntimeError(f"axon_start_nrt_profile rc={rc}")
            try:
                yield
            finally:
                n = lib.axon_stop_nrt_profile(str(output_dir).encode())
                print(f"ntff profile: {n} file(s) -> {output_dir}", file=sys.stderr)

        return _hook

    hook = _mk("/opt/axon/libaxon_pjrt.so")
    mod = types.ModuleType("antenv.axon_hooks")
    mod.get_axon_ntff_profile_hook = lambda: hook
    try:
        import antenv

        antenv.axon_hooks = mod
    except ImportError:
        pass
    sys.modules["antenv.axon_hooks"] = mod

    import concourse.bass_utils as bass_utils

    bass_utils.upload_artifacts = lambda tmpdir: f"local://{tmpdir}"


def _re3(ap):
    """[128, n*E] -> [128, n, E] view."""
    return ap.rearrange("p (n e) -> p n e", e=E)


def _re4(ap):
    """[128, NT*T] -> [128, NT, T] view."""
    return ap.rearrange("p (u v) -> p u v", v=T)


def build_program():
    nc = bass.Bass()

    adjT = nc.declare_dram_parameter("adjT", [G, 384, TOK], BF, isOutput=False)
    adjF8_d = nc.declare_dram_parameter("adjF8", [G, 128, 2, TOK], F8, isOutput=False)
    onT = nc.declare_dram_parameter("onT", [G, 384, N], BF, isOutput=False)
    quesT = nc.declare_dram_parameter("quesT", [G, 128, 4], BF, isOutput=False)
    w1a_d = nc.declare_dram_parameter("w1a", [128, 3, H], BF, isOutput=False)
    w1bDR_d = nc.declare_dram_parameter("w1bDR", [128, 2, H], F8, isOutput=False)
    w1b44_d = nc.declare_dram_parameter("w1b44", [44, H], BF, isOutput=False)
    w2aDR_d = nc.declare_dram_parameter("w2aDR", [128, 2, H], F8, isOutput=False)
    w2a44_d = nc.declare_dram_parameter("w2a44", [44, H], BF, isOutput=False)
    w2bDR_d = nc.declare_dram_parameter("w2bDR", [128, 4, H], F8, isOutput=False)
    wadj_d = nc.declare_dram_parameter("wadj", [128, 3, H], BF, isOutput=False)
    wq_d = nc.declare_dram_parameter("wq", [128, 4, H], BF, isOutput=False)
    we_d = nc.declare_dram_parameter("we", [128, 4, H], BF, isOutput=False)
    wv_d = nc.declare_dram_parameter("wv", [128, 4, H], BF, isOutput=False)
    # biases packed [128, 5*4]: bq|be|b2s|bv|badj, column j = chans j*128..
    bias_d = nc.declare_dram_parameter("bias", [128, 20], F32, isOutput=False)
    b1row_d = nc.declare_dram_parameter("b1row", [1, H], BF, isOutput=False)
    bqrow_d = nc.declare_dram_parameter("bqrow", [1, H], F32, isOutput=False)
    smat_d = nc.declare_dram_parameter("smat", [N + 1, TOK], BF, isOutput=False)

    outT = nc.declare_dram_parameter("outT", [G, 4, 128, N], F32, isOutput=True)
    qscr_d = nc.declare_dram_parameter("qscr", [G, H], F32, isOutput=True)

    def tsl(t):
        return slice(t * T, (t + 1) * T)

    with tile.TileContext(nc) as tc, contextlib.ExitStack() as ctx:
        wpool = ctx.enter_context(tc.tile_pool(name="weights", bufs=1))
        gpool = ctx.enter_context(tc.tile_pool(name="group", bufs=2))
        gpool3 = ctx.enter_context(tc.tile_pool(name="group3", bufs=3))
        spool = ctx.enter_context(tc.tile_pool(name="small", bufs=2))
        # 2-bank PSUM tiles [128, 2, 512] for the big stages (3 in flight)
        ps2 = ctx.enter_context(tc.tile_pool(name="ps2", bufs=3, space="PSUM"))
        # 1-bank tiles for warmup / q / ontT
        ps1 = ctx.enter_context(tc.tile_pool(name="ps1", bufs=2, space="PSUM"))

        # PE warmup: keep the HAM clock-gate at 8/8 through the startup
        # DMA wait so the first real matmuls run at 2.4 GHz.
        wu_sb = wpool.tile([128, 512], BF, tag="wu", name="wu")
        nc.vector.memset(wu_sb[:], 0.0)
        wu_ps = ps1.tile([128, T], F32, tag="ps1", name="wups")
        for i in range(85):
            nc.tensor.matmul(
                wu_ps[:], wu_sb[:, :128], wu_sb[:, :T], start=True, stop=True
            )

        def load_w_multi(dram, nchunks, chunks, name):
            t_ = wpool.tile([128, nchunks, H], BF, tag=name, name=name)
            nc.scalar.dma_start(out=t_[:], in_=dram[:, :, :])
            return [t_[: k1 - k0, ki, :] for ki, (k0, k1) in enumerate(chunks)]

        w1a_sb = load_w_multi(w1a_d, 3, KD, "w1a")
        wadj_sb = load_w_multi(wadj_d, 3, KD, "wadj")
        wq_sb = load_w_multi(wq_d, 4, KH, "wq")
        we_sb = load_w_multi(we_d, 4, KH, "we")
        wv_sb = load_w_multi(wv_d, 4, KH, "wv")

        w1bDR_sb = wpool.tile([128, 2, H], F8, tag="w1bDR", name="w1bDR")
        nc.scalar.dma_start(out=w1bDR_sb[:], in_=w1bDR_d[:, :, :])
        w2aDR_sb = wpool.tile([128, 2, H], F8, tag="w2aDR", name="w2aDR")
        nc.scalar.dma_start(out=w2aDR_sb[:], in_=w2aDR_d[:, :, :])
        w2bDR_sb = wpool.tile([128, 4, H], F8, tag="w2bDR", name="w2bDR")
        nc.scalar.dma_start(out=w2bDR_sb[:], in_=w2bDR_d[:, :, :])
        w2a44_sb = wpool.tile([44, H], BF, tag="w2a44", name="w2a44")
        nc.scalar.dma_start(out=w2a44_sb[:], in_=w2a44_d[:, :])

        bqrow_sb = wpool.tile([1, H], F32, tag="bqrow", name="bqrow")
        nc.scalar.dma_start(out=bqrow_sb[:], in_=bqrow_d[:, :])
        bias_sb = wpool.tile([128, 20], F32, tag="bias", name="bias")
        nc.scalar.dma_start(out=bias_sb[:], in_=bias_d[:, :])
        bq_sb = bias_sb[:, 0:4]
        be_sb = bias_sb[:, 4:8]
        b2s_sb = bias_sb[:, 8:12]
        bv_sb = bias_sb[:, 12:16]
        badj_sb = bias_sb[:, 16:20]

        def emit_preamble(g):
            """Loads + q-fold + on-term (+ its E-expansion)."""
            st = {}
            ques_sb = spool.tile([128, 4], BF, tag="ques", name=f"ques_{g}")
            nc.sync.dma_start(out=ques_sb[:], in_=quesT[g, :, :])
            adj3_sb = gpool3.tile([128, 3, TOK], BF, tag="adj3", name=f"adj3_{g}")
            nc.sync.dma_start(
                out=adj3_sb[:],
                in_=adjT[g].rearrange("(s p) t -> p s t", p=128),
            )
            adj_sb = [adj3_sb[: k1 - k0, ki, :] for ki, (k0, k1) in enumerate(KD)]
            adjF8_sb = gpool3.tile([128, 2, TOK], F8, tag="adjF8", name=f"adjF8_{g}")
            nc.sync.dma_start(out=adjF8_sb[:], in_=adjF8_d[g, :, :, :])
            on3_sb = spool.tile([128, 3, N], BF, tag="on3", name=f"on3_{g}")
            nc.sync.dma_start(
                out=on3_sb[:],
                in_=onT[g].rearrange("(s p) n -> p s n", p=128),
            )
            on_sb = [on3_sb[: k1 - k0, ki, :] for ki, (k0, k1) in enumerate(KD)]
            st["adj"] = adj_sb
            st["adjF8"] = adjF8_sb

            # q^T = ques^T @ Wq + bq, as 4 wide matmuls (M=1, N=512) —
            # avoids 16 single-column matmuls and their weight reloads.
            q_ps = ps1.tile([1, H], F32, tag="ps1", name=f"qps_{g}")
            for k in range(4):
                nc.tensor.matmul(
                    q_ps[:],
                    ques_sb[:, k : k + 1],
                    wq_sb[k][:],
                    start=(k == 0),
                    stop=(k == 3),
                )
            qrow = spool.tile([1, H], F32, tag="qrow", name=f"qrow_{g}")
            nc.vector.tensor_tensor(
                qrow[:], q_ps[:], bqrow_sb[:], op=ALU.add
            )
            # redistribute [1, 512] -> [128, 4] (channel j*128+p -> (p, j))
            # via a DRAM bounce: SBUF->SBUF can't scatter one partition's
            # row across partitions, DRAM APs are unconstrained.
            nc.sync.dma_start(out=qscr_d[g, :], in_=qrow[:])
            q_sb = spool.tile([128, 4], F32, tag="q", name=f"q_{g}")
            nc.sync.dma_start(
                out=q_sb[:],
                in_=qscr_d[g].rearrange("(j p) -> p j", p=128),
            )

            # fold q into We, Wv (host pre-scaled x32) -> fp8 DR layout.
            # For the first two groups the scalar engine is the startup
            # critical path (ontT/w1x chain), so fold on the idle DVE there.
            weq_sb = gpool3.tile([128, 4, H], F8, tag="weq", name=f"weq_{g}")
            wvq_sb = gpool3.tile([128, 4, H], F8, tag="wvq", name=f"wvq_{g}")
            for dst, wsrc in ((weq_sb, we_sb), (wvq_sb, wv_sb)):
                for k in range(4):
                    if g < 2:
                        nc.vector.tensor_scalar_mul(
                            dst[:, k, :], wsrc[k][:], q_sb[:, k : k + 1]
                        )
                    else:
                        nc.scalar.activation(
                            out=dst[:, k, :], in_=wsrc[k][:], func=ACTF.Copy,
                            scale=q_sb[:, k : k + 1],
                        )
            st["weq"] = weq_sb
            st["wvq"] = wvq_sb

            # transposed on-term: ontT[n, c] = sum_f on[f, n] * W1a[f, c]
            # (lhsT = on chunk, rhs = W1a chunk -- no transpose needed).
            # Packed stage-A operands: one K=125 matmul covers
            # [ontT (80) | b1 (1) | w1b chunk3 (44)] against
            # [smat (80) | ones (1) | adj chunk3 (44)], folding the
            # on-term + bias + ragged D-chunk into a single accumulation.
            # All three row groups carry the x32 fp8 scale.
            ontT_ps = ps1.tile([N, H], F32, tag="ps1", name=f"ontTps_{g}")
            for ki in range(3):
                nc.tensor.matmul(
                    ontT_ps[:],
                    on_sb[ki][:],
                    w1a_sb[ki][:],
                    start=(ki == 0),
                    stop=(ki == 2),
                )
            KX = N + 1 + (D - 256)  # 125
            w1x_sb = spool.tile([KX, H], BF, tag="w1x", name=f"w1x_{g}")
            nc.scalar.activation(
                out=w1x_sb[:N, :], in_=ontT_ps[:], func=ACTF.Copy, scale=SC
            )
            nc.sync.dma_start(out=w1x_sb[N : N + 1, :], in_=b1row_d[:, :])
            nc.sync.dma_start(out=w1x_sb[N + 1 :, :], in_=w1b44_d[:, :])
            adjx_sb = gpool3.tile([KX, TOK], BF, tag="adjx", name=f"adjx_{g}")
            nc.sync.dma_start(out=adjx_sb[: N + 1, :], in_=smat_d[:, :])
            nc.sync.dma_start(out=adjx_sb[N + 1 :, :], in_=adjT[g, 256:D, :])
            st["w1x"] = w1x_sb
            st["adjx"] = adjx_sb
            return st

        def ps_pair(name):
            pa = ps2.tile([128, 2, 512], F32, tag="ps2", name=f"{name}a")
            pb = ps2.tile([128, 2, 512], F32, tag="ps2", name=f"{name}b")
            return pa, pb

        def pst(pa, pb, t):
            return (pa if t < 2 else pb)[:, t % 2, 0:T]

        def emit_AB(g, st):
            """Stage A/B + softmax-a chain (C/D)."""
            adj_sb = st["adj"]
            adjF8_sb = st["adjF8"]
            w1x_sb = st["w1x"]
            adjx_sb = st["adjx"]
            weq_sb = st["weq"]

            # edges (x32, fp8, [128, mchunk, tok]); expa = exp(logits_a) bf16
            edges_sb = gpool.tile([128, 4, TOK], F8, tag="edges", name=f"edges_{g}")
            expa_sb = gpool.tile([128, 4, TOK], BF, tag="expa", name=f"expa_{g}")
            st["edges"] = edges_sb
            st["expa"] = expa_sb

            # stage A: 32*edges = (32 W1b[0:256]) @ adj  (fp8 DoubleRow)
            #        + [32 ontT | 32 b1 | 32 W1b44] @ [smat | ones | adj44]
            for m, (m0, m1) in enumerate(MS):
                pa, pb = ps_pair(f"eps_{g}_{m}")
                # snake the two weight groups so adjacent m-blocks share a
                # boundary stationary operand (dedupe drops the reload)
                parts = [
                    lambda s, e: [
                        nc.tensor.matmul(
                            pst(pa, pb, t), w1bDR_sb[:, :, m0:m1],
                            adjF8_sb[:, :, tsl(t)],
                            start=s, stop=e, perf_mode=DR,
                        )
                        for t in range(NT)
                    ],
                    lambda s, e: [
                        nc.tensor.matmul(
                            pst(pa, pb, t), w1x_sb[:, m0:m1],
                            adjx_sb[:, tsl(t)], start=s, stop=e,
                        )
                        for t in range(NT)
                    ],
                ]
                if m % 2:
                    parts.reverse()
                parts[0](True, False)
                parts[1](False, True)
                ev = _re4(edges_sb[:, m, :])
                nc.scalar.copy(out=ev[:, 0:2, :], in_=pa[:, :, 0:T])
                nc.scalar.copy(out=ev[:, 2:4, :], in_=pb[:, :, 0:T])

            # stage B: expa = exp((1/1024) * (32 We q) @ (32 edges) + be)
            e3 = edges_sb
            for m, (m0, m1) in enumerate(MS):
                pa, pb = ps_pair(f"lps_{g}_{m}")
                order = (0, 1) if m % 2 == 0 else (1, 0)
                for j, i in enumerate(order):
                    for t in range(NT):
                        nc.tensor.matmul(
                            pst(pa, pb, t),
                            weq_sb[:, 2 * i : 2 * i + 2, m0:m1],
                            e3[:, 2 * i : 2 * i + 2, tsl(t)],
                            start=(j == 0),
                            stop=(j == 1),
                            perf_mode=DR,
                        )
                ea = _re4(expa_sb[:, m, :])
                nc.scalar.activation(
                    out=ea[:, 0:2, :], in_=pa[:, :, 0:T], func=ACTF.Exp,
                    bias=be_sb[:, m : m + 1], scale=ISC2,
                )
                nc.scalar.activation(
                    out=ea[:, 2:4, :], in_=pb[:, :, 0:T], func=ACTF.Exp,
                    bias=be_sb[:, m : m + 1], scale=ISC2,
                )

            # stage C: suma, reca = 1/suma (fast-approx NR reciprocal)
            suma = spool.tile([128, 4, N], F32, tag="suma", name=f"suma_{g}")
            for m in range(4):
                nc.vector.tensor_reduce(
                    suma[:, m, :], _re3(expa_sb[:, m, :]), axis=AX.X, op=ALU.add
                )
            reca = spool.tile([128, 4, N], F32, tag="reca", name=f"reca_{g}")
            nc.vector.reciprocal(reca[:], suma[:])

            # stage D: a = expa * reca (in place, gpsimd);
            # edges2 = a * (32 edges) -> fp8 (gpsimd; DVE pays a ~2x
            # penalty on fp8-operand tensor_tensor)
            edges2_sb = gpool.tile([128, 4, TOK], F8, tag="e2", name=f"e2_{g}")
            st["edges2"] = edges2_sb
            for m in range(4):
                eam = _re3(expa_sb[:, m, :])
                rb = reca[:, m, :]
                nc.gpsimd.tensor_tensor(
                    out=eam,
                    in0=eam,
                    in1=rb[:, :, None].broadcast_to((128, N, E)),
                    op=ALU.mult,
                )
                e2eng = nc.vector if (g == 0 and m >= 2) else nc.gpsimd
                e2eng.tensor_tensor(
                    out=edges2_sb[:, m, :],
                    in0=expa_sb[:, m, :],
                    in1=edges_sb[:, m, :],
                    op=ALU.mult,
                )
            return st

        def emit_EF(g, st):
            """Stages E and F for group g."""
            adj_sb = st["adj"]
            adjF8_sb = st["adjF8"]
            edges2_sb = st["edges2"]
            wvq_sb = st["wvq"]

            t_sb = gpool.tile([128, 4, TOK], F8, tag="tt", name=f"t_{g}")
            expb_sb = gpool.tile([128, 4, TOK], BF, tag="expb", name=f"expb_{g}")
            st["expb"] = expb_sb

            # stage E: 32*t = (32 W2a[0:256]) @ adj (DR) + (32 W2a44) @ adj44
            #        + W2b @ (32 edges2) (DR); +32*b2 at the drain
            for m, (m0, m1) in enumerate(MS):
                pa, pb = ps_pair(f"tps_{g}_{m}")
                def mk_dr(wslice, rslice_t):
                    def emit(s, e):
                        for t in range(NT):
                            nc.tensor.matmul(
                                pst(pa, pb, t), wslice, rslice_t(t),
                                start=s, stop=e, perf_mode=DR,
                            )
                    return emit
                def mk_bf(wslice, rslice_t):
                    def emit(s, e):
                        for t in range(NT):
                            nc.tensor.matmul(
                                pst(pa, pb, t), wslice, rslice_t(t),
                                start=s, stop=e,
                            )
                    return emit
                parts = [
                    mk_dr(w2aDR_sb[:, :, m0:m1],
                          lambda t: adjF8_sb[:, :, tsl(t)]),
                    mk_dr(w2bDR_sb[:, 0:2, m0:m1],
                          lambda t: edges2_sb[:, 0:2, tsl(t)]),
                    mk_dr(w2bDR_sb[:, 2:4, m0:m1],
                          lambda t: edges2_sb[:, 2:4, tsl(t)]),
                    mk_bf(w2a44_sb[:, m0:m1],
                          lambda t: adj_sb[2][:, tsl(t)]),
                ]
                if m % 2:
                    parts.reverse()
                for j, p in enumerate(parts):
                    p(j == 0, j == len(parts) - 1)
                tv = _re4(t_sb[:, m, :])
                nc.scalar.activation(
                    out=tv[:, 0:2, :], in_=pa[:, :, 0:T], func=ACTF.Identity,
                    bias=b2s_sb[:, m : m + 1],
                )
                nc.scalar.activation(
                    out=tv[:, 2:4, :], in_=pb[:, :, 0:T], func=ACTF.Identity,
                    bias=b2s_sb[:, m : m + 1],
                )

            # stage F: expb = exp((1/1024) * (32 Wv q) @ (32 t) + bv)
            for m, (m0, m1) in enumerate(MS):
                pa, pb = ps_pair(f"bps_{g}_{m}")
                order = (0, 1) if m % 2 == 0 else (1, 0)
                for j, i in enumerate(order):
                    for t in range(NT):
                        nc.tensor.matmul(
                            pst(pa, pb, t),
                            wvq_sb[:, 2 * i : 2 * i + 2, m0:m1],
                            t_sb[:, 2 * i : 2 * i + 2, tsl(t)],
                            start=(j == 0),
                            stop=(j == 1),
                            perf_mode=DR,
                        )
                eb = _re4(expb_sb[:, m, :])
                nc.scalar.activation(
                    out=eb[:, 0:2, :], in_=pa[:, :, 0:T], func=ACTF.Exp,
                    bias=bv_sb[:, m : m + 1], scale=ISC2,
                )
                nc.scalar.activation(
                    out=eb[:, 2:4, :], in_=pb[:, :, 0:T], func=ACTF.Exp,
                    bias=bv_sb[:, m : m + 1], scale=ISC2,
                )

        def emit_GHI(g, st):
            """Stages G..I for group g."""
            adj_sb = st["adj"]
            expb_sb = st["expb"]

            # stage G: sumb, recb
            sumb = spool.tile([128, 4, N], F32, tag="sumb", name=f"sumb_{g}")
            for m in range(4):
                nc.vector.tensor_reduce(
                    sumb[:, m, :], _re3(expb_sb[:, m, :]), axis=AX.X, op=ALU.add
                )
            recb = spool.tile([128, 4, N], F32, tag="recb", name=f"recb_{g}")

            # stage H: pre = (Wadj @ adj + badj) * expb  (into expb_sb, bf16)
            for m, (m0, m1) in enumerate(MS):
                pa, pb = ps_pair(f"aps_{g}_{m}")
                korder = (0, 1, 2) if m % 2 == 0 else (2, 1, 0)
                for j, ki in enumerate(korder):
                    for t in range(NT):
                        nc.tensor.matmul(
                            pst(pa, pb, t),
                            wadj_sb[ki][:, m0:m1],
                            adj_sb[ki][:, tsl(t)],
                            start=(j == 0),
                            stop=(j == 2),
                        )
                eb = _re4(expb_sb[:, m, :])
                nc.vector.scalar_tensor_tensor(
                    out=eb[:, 0:2, :],
                    in0=pa[:, :, 0:T],
                    scalar=badj_sb[:, m : m + 1],
                    in1=eb[:, 0:2, :],
                    op0=ALU.add,
                    op1=ALU.mult,
                )
                nc.vector.scalar_tensor_tensor(
                    out=eb[:, 2:4, :],
                    in0=pb[:, :, 0:T],
                    scalar=badj_sb[:, m : m + 1],
                    in1=eb[:, 2:4, :],
                    op0=ALU.add,
                    op1=ALU.mult,
                )

            # stage I: out = (sum_E pre) * recb ; store
            s_sb = spool.tile([128, 4, N], F32, tag="s", name=f"s_{g}")
            for m in range(4):
                nc.vector.tensor_reduce(
                    s_sb[:, m, :], _re3(expb_sb[:, m, :]), axis=AX.X, op=ALU.add
                )
            # reciprocal sits after the PSUM-freeing drains on the DVE queue
            nc.vector.reciprocal(recb[:], sumb[:])
            o_sb = spool.tile([128, 4, N], F32, tag="o", name=f"o_{g}")
            nc.gpsimd.tensor_tensor(
                out=o_sb[:], in0=s_sb[:], in1=recb[:], op=ALU.mult
            )
            nc.sync.dma_start(
                out=outT[g].rearrange("f p n -> p f n"), in_=o_sb[:]
            )

        # software pipeline across groups, 3 stages deep:
        #   ... AB(g) | EF(g-1) | preamble(g+1) | GHI(g-1) ...
        # The serial preamble chain (q -> weight folds -> on-term expand) of
        # g+1 is tucked after stage F's ACT work so it never sits between
        # stage-critical ACT/DVE ops, and completes long before AB(g+1).
        states = {0: emit_preamble(0), 1: emit_preamble(1)}
        for g in range(G):
            emit_AB(g, states[g])
            if g >= 1:
                emit_EF(g - 1, states[g - 1])
            if g >= 1 and g + 1 < G:
                states[g + 1] = emit_preamble(g + 1)
            if g >= 1:
                emit_GHI(g - 1, states.pop(g - 1))
        emit_EF(G - 1, states[G - 1])
        emit_GHI(G - 1, states.pop(G - 1))

    nsplit = _split_multi_waits(nc)
    if os.environ.get("KERNEL_DEBUG"):
        print(f"split_multi_waits: {nsplit} nops inserted", file=sys.stderr)
    return nc


def _pack_bias(b):
    # [H] -> [128, 4]: column j = channels j*128..(j+1)*128
    return np.ascontiguousarray(np.asarray(b, np.float32).reshape(4, 128).T)


def _bf(x):
    return np.ascontiguousarray(np.asarray(x, np.float32).astype(ml_dtypes.bfloat16))


def _f8(x):
    x = np.clip(np.asarray(x, np.float32), -240.0, 240.0)
    return np.ascontiguousarray(x.astype(ml_dtypes.float8_e4m3))


def _smat():
    """[N+1, TOK] node->token selection matrix (+ ones row for the b1 bias)."""
    s = np.zeros((N + 1, TOK), np.float32)
    for n in range(N):
        s[n, n * E : (n + 1) * E] = 1.0
    s[N, :] = 1.0
    return _bf(s)


def prepare_inputs(ques_embed, adj_list, original_nodes,
                   w1_w, w1_b, wq_w, wq_b, we_w, we_b,
                   w2_w, w2_b, wv_w, wv_b, wadj_w, wadj_b):
    """Host-side layout prep: feature-major tensors, fp8 DoubleRow weight
    layouts (x32), plus per-core shards. Returns per-core input maps."""
    adjTf = np.asarray(adj_list, np.float32).reshape(BR, TOK, D).transpose(0, 2, 1)
    adjT = _bf(np.concatenate(
        [adjTf, np.zeros((BR, 384 - D, TOK), np.float32)], axis=1))
    # fp8 copy of adj rows 0..255, DoubleRow layout [128, 2, TOK]
    adjF8 = _f8(adjTf[:, :256, :].reshape(BR, 2, 128, TOK).transpose(0, 2, 1, 3))
    onTf = np.asarray(original_nodes, np.float32).reshape(BR, N, D).transpose(0, 2, 1)
    onT = _bf(np.concatenate(
        [onTf, np.zeros((BR, 384 - D, N), np.float32)], axis=1))
    quesT = _bf(
        np.asarray(ques_embed, np.float32).reshape(BR, 4, 128).transpose(0, 2, 1)
    )

    w1b = np.asarray(w1_w, np.float32)[:, D:].T    # [D, H]
    w2a = np.asarray(w2_w, np.float32)[:, :D].T    # [D, H]
    w2b = np.asarray(w2_w, np.float32)[:, D:].T    # [H, H]

    def _dr2(w):
        # [256, H] -> [128, 2, H] DoubleRow pairing (slot i = rows 128i..)
        return w[:256].reshape(2, 128, H).transpose(1, 0, 2)

    def _chunk3(wdh):
        # [D, H] -> [128, 3, H] (slot s = rows 128s.., zero-padded)
        p = np.concatenate([np.asarray(wdh, np.float32),
                            np.zeros((384 - D, H), np.float32)], axis=0)
        return p.reshape(3, 128, H).transpose(1, 0, 2)

    def _chunk4(whh):
        return np.asarray(whh, np.float32).reshape(4, 128, H).transpose(1, 0, 2)

    w = {
        "w1a": _bf(_chunk3(np.asarray(w1_w)[:, :D].T)),
        "w1bDR": _f8(SC * _dr2(w1b)),
        "w1b44": _bf(SC * w1b[256:D]),
        "w2aDR": _f8(SC * _dr2(w2a)),
        "w2a44": _bf(SC * w2a[256:D]),
        "w2bDR": _f8(w2b.reshape(4, 128, H).transpose(1, 0, 2)),
        "wadj": _bf(_chunk3(np.asarray(wadj_w).T)),
        "wq": _bf(_chunk4(np.asarray(wq_w).T)),
        "we": _bf(_chunk4(SC * np.asarray(we_w, np.float32).T)),
        "wv": _bf(_chunk4(SC * np.asarray(wv_w, np.float32).T)),
        "bias": np.ascontiguousarray(np.concatenate([
            _pack_bias(wq_b), _pack_bias(we_b),
            _pack_bias(SC * np.asarray(w2_b, np.float32)),
            _pack_bias(wv_b), _pack_bias(wadj_b)], axis=1)),
        "b1row": _bf(SC * np.asarray(w1_b, np.float32).reshape(1, H)),
        "bqrow": np.ascontiguousarray(
            np.asarray(wq_b, np.float32).reshape(1, H)),
        "smat": _smat(),
    }

    in_maps = []
    for c in range(NCORES):
        sl = slice(c * G, (c + 1) * G)
        m = dict(w)
        m["adjT"] = np.ascontiguousarray(adjT[sl])
        m["adjF8"] = np.ascontiguousarray(adjF8[sl])
        m["onT"] = np.ascontiguousarray(onT[sl])
        m["quesT"] = np.ascontiguousarray(quesT[sl])
        in_maps.append(m)
    return in_maps


def run(in_maps, trace=False, tmpdir=None):
    _install_ntff_hook()
    if not os.environ.get("KERNEL_NO_LDW_DEDUPE"):
        _patch_ldw_dedupe()
    from concourse.bass_utils import run_bass_kernel_spmd

    nc = build_program()
    res = run_bass_kernel_spmd(
        nc,
        in_maps,
        core_ids=list(range(NCORES)),
        trace=trace,
        tmpdir=tmpdir,
    )
    return res


def gather_output(res):
    outT = np.stack([res.results[c]["outT"] for c in range(NCORES)])  # [8,5,4,128,N]
    outT = outT.reshape(BR, 4, 128, N).transpose(0, 3, 1, 2)          # [40,N,4,128]
    return np.ascontiguousarray(outT.reshape(B, R, N, H).astype(np.float32))


def kernel(ques_embed, adj_list, original_nodes,
           w1_w, w1_b, wq_w, wq_b, we_w, we_b,
           w2_w, w2_b, wv_w, wv_b, wadj_w, wadj_b,
           deg=None, batch_size=None, **_unused):
    in_maps = prepare_inputs(
        ques_embed, adj_list, original_nodes,
        w1_w, w1_b, wq_w, wq_b, we_w, we_b,
        w2_w, w2_b, wv_w, wv_b, wadj_w, wadj_b,
    )
    res = run(in_maps, trace=False)
    return gather_output(res)


# revision 26
# speedup vs baseline: 1.0087x; 1.0087x over previous
"""Trainium2 Bass kernel for nn_MessagePassing (gnn_message_passing).

Self-contained: takes full (unsharded) numpy inputs, shards batch*rounds
across 8 NeuronCores, runs a Bass/Tile kernel per core, gathers the full
output.

Math (per (b,r) group, all biases included):
  q      = Wq @ ques + bq                       [H]
  edges  = W1a @ on + W1b @ adj + b1            [H, N*E]  (on broadcast over E)
  a      = softmax_E(We @ (q*edges) + be)       -> folded:  (We*diag(q)) @ edges
  edges2 = a * edges
  t      = W2a @ adj + W2b @ edges2 + b2
  b      = softmax_E(Wv @ (q*t) + bv)           -> folded:  (Wv*diag(q)) @ t
  out    = sum_E b * (Wadj @ adj + badj)        [H, N]

Precision scheme: the softmax-logit paths (stages A/B/E/F) run as fp8
DoubleRow matmuls (2 contraction rows per PE cell, ~1.8x streaming rate).
fp8 noise there is damped by the near-uniform softmaxes (logit std ~0.1),
so it barely reaches the output. The direct value path (Wadj @ adj, stage
H) stays bf16. fp8 weights are scaled x32 so w*32 ~ N(0, 0.64) clears the
e4m3 subnormal floor; intermediates (edges, t, edges2) are stored x32 in
fp8 and the exp() activations unscale by 1/1024 (= 1/32 * 1/32).

Layout on device: hidden channels on partitions (4 chunks of 128), tokens
(node*E+e) on the free dim, so softmax over E is a free-dim segment reduce.
PSUM is used as 2-bank [128, 2, 512] tiles so each stage drain is one
strided DVE/ACT instruction over 800 tokens instead of two 400-token ones.

Schedule: groups are software-pipelined — the front half (loads, q-fold,
edges, softmax-a chain) of group g is emitted before the back half
(t/softmax-b/output) of group g-1, so the PE always has a ready matmul
stage while the DVE/ACT/GPSIMD softmax chain of the newer group runs.
"""

import os
import sys

for _p in ("/opt/trn_rl_repo", "/root/.axon_site/_ro/trn_rl_repo",
           "/root/.axon_site/_ro/pypackages"):
    if _p not in sys.path and os.path.isdir(_p):
        sys.path.append(_p)

import contextlib
import ctypes
import types

import ml_dtypes
import numpy as np

import concourse.bass as bass
import concourse.tile as tile
from concourse import mybir

BF = mybir.dt.bfloat16
F32 = mybir.dt.float32
F16 = mybir.dt.float16
F8 = mybir.dt.float8e4
AX = mybir.AxisListType
ALU = mybir.AluOpType
ACTF = mybir.ActivationFunctionType
DR = mybir.MatmulPerfMode.DoubleRow

B, R, N, E, D, H = 4, 10, 80, 20, 300, 512
BR = B * R              # 40 (b,r) groups
NCORES = 8
G = BR // NCORES        # 5 groups per core
TOK = N * E             # 1600 tokens per group
NT = 4                  # token tiles per group
T = TOK // NT           # 400 tokens per tile
TN = N // NT            # 20 nodes per tile

KD = [(0, 128), (128, 256), (256, 300)]               # D=300 contraction chunks
KH = [(0, 128), (128, 256), (256, 384), (384, 512)]   # H=512 contraction chunks
MS = [(0, 128), (128, 256), (256, 384), (384, 512)]   # output chunks

SC = 32.0               # fp8 weight/intermediate scale
ISC2 = 1.0 / (SC * SC)  # unscale for exp() after (32w) @ (32x)

_MAXW = 1  # this walrus build allows a single semaphore wait per instruction


def _split_multi_waits(nc):
    """Walrus here rejects instructions with >1 sem wait; hoist extra waits
    onto same-engine NoOps inserted just before the instruction."""
    ctr = 0
    for fn in nc.m.functions:
        for bb in fn.blocks:
            new = []
            for inst in bb.instructions:
                si = inst.sync_info
                if si is not None:
                    waits = list(si.on_wait)
                    if len(waits) > _MAXW:
                        for i in range(0, len(waits) - _MAXW, _MAXW):
                            ctr += 1
                            nop = mybir.InstNoOp(name=f"wsplit-{ctr}")
                            nop.engine = inst.engine
                            nop.sync_info = mybir.SyncInfo(
                                on_wait=waits[i : i + _MAXW], on_update=[]
                            )
                            new.append(nop)
                        si.on_wait = waits[len(waits) - _MAXW :]
                new.append(inst)
            bb.instructions = new
    return ctr


def _patch_ldw_dedupe():
    """The bass pipeline splits every matmul into Ldweights + Matmult.
    Consecutive matmuls that share the stationary operand then reload the
    same weights. Drop the redundant Ldweights at the BIR-JSON level
    (walrus's own --enable-ldw-opt rejects explicit Ldweights)."""
    import orjson

    import concourse.bass2jax as b2j
    import concourse.bass_utils as bu

    if getattr(bu, "_ldw_dedupe_patched", False):
        return
    orig = bu.compile_bir_kernel

    def _dedupe(bir_json):
        d = orjson.loads(bir_json)
        removed = 0
        nopctr = 0
        for fn in d.get("functions", []):
            stack = list(fn.get("blocks", []))
            while stack:
                blk = stack.pop()
                stack.extend(blk.get("blocks", []))
                insts = blk.get("instructions", [])
                out = []
                last_key = None
                for i in insts:
                    op = i.get("opcode")
                    if op == "Ldweights":
                        key = orjson.dumps(
                            [
                                i.get("ins"),
                                i.get("perf_mode"),
                                i.get("tile_position"),
                                i.get("tile_size"),
                                i.get("is_transpose"),
                            ]
                        )
                        si = i.get("sync_info") or {}
                        if key == last_key and not si.get("on_update"):
                            w = si.get("on_wait") or []
                            if w:
                                nopctr += 1
                                out.append(
                                    {
                                        "name": f"ldwkeep-{nopctr}",
                                        "opcode": "NoOp",
                                        "engine": i.get("engine", "PE"),
                                        "ins": [],
                                        "outs": [],
                                        "sync_info": {
                                            "on_wait": w,
                                            "on_update": [],
                                        },
                                    }
                                )
                            removed += 1
                            continue
                        last_key = key
                    elif op == "Matmult":
                        if i.get("is_transpose") or i.get("ldweights"):
                            last_key = None
                    out.append(i)
                blk["instructions"] = out
        if os.environ.get("KERNEL_DEBUG"):
            print(f"ldw dedupe: removed {removed}", file=sys.stderr)
        return orjson.dumps(d)

    def compile_bir_kernel(bir_json, tmpdir, neff_name="file.neff"):
        try:
            bir_json = _dedupe(bir_json)
        except Exception as e:  # pragma: no cover - safety net
            print(f"ldw dedupe skipped: {e}", file=sys.stderr)
        return orig(bir_json, tmpdir, neff_name=neff_name)

    bu.compile_bir_kernel = compile_bir_kernel
    b2j.compile_bir_kernel = compile_bir_kernel
    bu._ldw_dedupe_patched = True


def _install_ntff_hook():
    """Provide antenv.axon_hooks (missing in this image) so that
    run_bass_kernel_spmd(trace=True) can profile via libaxon_pjrt."""
    if "antenv.axon_hooks" in sys.modules:
        return

    def _mk(so_path):
        try:
            lib = ctypes.CDLL(so_path)
        except OSError:
            return None
        if not hasattr(lib, "axon_start_nrt_profile"):
            return None
        lib.axon_start_nrt_profile.argtypes = [
            ctypes.POINTER(ctypes.c_int64),
            ctypes.c_size_t,
        ]
        lib.axon_start_nrt_profile.restype = ctypes.c_int64
        lib.axon_stop_nrt_profile.argtypes = [ctypes.c_char_p]
        lib.axon_stop_nrt_profile.restype = ctypes.c_int64

        @contextlib.contextmanager
        def _hook(output_dir, device_ids):
            import jax

            jax.devices()
            if device_ids:
                ids = (ctypes.c_int64 * len(device_ids))(*device_ids)
                rc = lib.axon_start_nrt_profile(ids, len(device_ids))
            else:
                rc = lib.axon_start_nrt_profile(None, 0)
            if rc != 0:
                raise RuntimeError(f"axon_start_nrt_profile rc={rc}")
            try:
                yield
            finally:
                n = lib.axon_stop_nrt_profile(str(output_dir).encode())
                print(f"ntff profile: {n} file(s) -> {output_dir}", file=sys.stderr)

        return _hook

    hook = _mk("/opt/axon/libaxon_pjrt.so")
    mod = types.ModuleType("antenv.axon_hooks")
    mod.get_axon_ntff_profile_hook = lambda: hook
    try:
        import antenv

        antenv.axon_hooks = mod
    except ImportError:
        pass
    sys.modules["antenv.axon_hooks"] = mod

    import concourse.bass_utils as bass_utils

    bass_utils.upload_artifacts = lambda tmpdir: f"local://{tmpdir}"


def _re3(ap):
    """[128, n*E] -> [128, n, E] view."""
    return ap.rearrange("p (n e) -> p n e", e=E)


def _re4(ap):
    """[128, NT*T] -> [128, NT, T] view."""
    return ap.rearrange("p (u v) -> p u v", v=T)


def build_program():
    nc = bass.Bass()

    adjT = nc.declare_dram_parameter("adjT", [G, 384, TOK], BF, isOutput=False)
    adjF8_d = nc.declare_dram_parameter("adjF8", [G, 128, 2, TOK], F8, isOutput=False)
    onT = nc.declare_dram_parameter("onT", [G, 384, N], BF, isOutput=False)
    quesT = nc.declare_dram_parameter("quesT", [G, 128, 4], BF, isOutput=False)
    w1a_d = nc.declare_dram_parameter("w1a", [128, 3, H], BF, isOutput=False)
    w1bDR_d = nc.declare_dram_parameter("w1bDR", [128, 2, H], F8, isOutput=False)
    w1b44_d = nc.declare_dram_parameter("w1b44", [44, H], BF, isOutput=False)
    w2aDR_d = nc.declare_dram_parameter("w2aDR", [128, 2, H], F8, isOutput=False)
    w2a44_d = nc.declare_dram_parameter("w2a44", [44, H], BF, isOutput=False)
    w2bDR_d = nc.declare_dram_parameter("w2bDR", [128, 4, H], F8, isOutput=False)
    wadj_d = nc.declare_dram_parameter("wadj", [128, 3, H], BF, isOutput=False)
    wq_d = nc.declare_dram_parameter("wq", [128, 4, H], BF, isOutput=False)
    we_d = nc.declare_dram_parameter("we", [128, 4, H], BF, isOutput=False)
    wv_d = nc.declare_dram_parameter("wv", [128, 4, H], BF, isOutput=False)
    # biases packed [128, 5*4]: bq|be|b2s|bv|badj, column j = chans j*128..
    bias_d = nc.declare_dram_parameter("bias", [128, 20], F32, isOutput=False)
    b1row_d = nc.declare_dram_parameter("b1row", [1, H], BF, isOutput=False)
    smat_d = nc.declare_dram_parameter("smat", [N + 1, TOK], BF, isOutput=False)

    outT = nc.declare_dram_parameter("outT", [G, 4, 128, N], F32, isOutput=True)

    def tsl(t):
        return slice(t * T, (t + 1) * T)

    with tile.TileContext(nc) as tc, contextlib.ExitStack() as ctx:
        wpool = ctx.enter_context(tc.tile_pool(name="weights", bufs=1))
        gpool = ctx.enter_context(tc.tile_pool(name="group", bufs=2))
        gpool3 = ctx.enter_context(tc.tile_pool(name="group3", bufs=3))
        spool = ctx.enter_context(tc.tile_pool(name="small", bufs=2))
        # 2-bank PSUM tiles [128, 2, 512] for the big stages (3 in flight)
        ps2 = ctx.enter_context(tc.tile_pool(name="ps2", bufs=3, space="PSUM"))
        # 1-bank tiles for warmup / q / ontT
        ps1 = ctx.enter_context(tc.tile_pool(name="ps1", bufs=2, space="PSUM"))

        # PE warmup: keep the HAM clock-gate at 8/8 through the startup
        # DMA wait so the first real matmuls run at 2.4 GHz.
        wu_sb = wpool.tile([128, 512], BF, tag="wu", name="wu")
        nc.vector.memset(wu_sb[:], 0.0)
        wu_ps = ps1.tile([128, T], F32, tag="ps1", name="wups")
        for i in range(85):
            nc.tensor.matmul(
                wu_ps[:], wu_sb[:, :128], wu_sb[:, :T], start=True, stop=True
            )

        def load_w_multi(dram, nchunks, chunks, name):
            t_ = wpool.tile([128, nchunks, H], BF, tag=name, name=name)
            nc.scalar.dma_start(out=t_[:], in_=dram[:, :, :])
            return [t_[: k1 - k0, ki, :] for ki, (k0, k1) in enumerate(chunks)]

        w1a_sb = load_w_multi(w1a_d, 3, KD, "w1a")
        wadj_sb = load_w_multi(wadj_d, 3, KD, "wadj")
        wq_sb = load_w_multi(wq_d, 4, KH, "wq")
        we_sb = load_w_multi(we_d, 4, KH, "we")
        wv_sb = load_w_multi(wv_d, 4, KH, "wv")

        w1bDR_sb = wpool.tile([128, 2, H], F8, tag="w1bDR", name="w1bDR")
        nc.scalar.dma_start(out=w1bDR_sb[:], in_=w1bDR_d[:, :, :])
        w2aDR_sb = wpool.tile([128, 2, H], F8, tag="w2aDR", name="w2aDR")
        nc.scalar.dma_start(out=w2aDR_sb[:], in_=w2aDR_d[:, :, :])
        w2bDR_sb = wpool.tile([128, 4, H], F8, tag="w2bDR", name="w2bDR")
        nc.scalar.dma_start(out=w2bDR_sb[:], in_=w2bDR_d[:, :, :])
        w2a44_sb = wpool.tile([44, H], BF, tag="w2a44", name="w2a44")
        nc.scalar.dma_start(out=w2a44_sb[:], in_=w2a44_d[:, :])

        bias_sb = wpool.tile([128, 20], F32, tag="bias", name="bias")
        nc.scalar.dma_start(out=bias_sb[:], in_=bias_d[:, :])
        bq_sb = bias_sb[:, 0:4]
        be_sb = bias_sb[:, 4:8]
        b2s_sb = bias_sb[:, 8:12]
        bv_sb = bias_sb[:, 12:16]
        badj_sb = bias_sb[:, 16:20]

        def emit_preamble(g):
            """Loads + q-fold + on-term (+ its E-expansion)."""
            st = {}
            ques_sb = spool.tile([128, 4], BF, tag="ques", name=f"ques_{g}")
            nc.sync.dma_start(out=ques_sb[:], in_=quesT[g, :, :])
            adj3_sb = gpool3.tile([128, 3, TOK], BF, tag="adj3", name=f"adj3_{g}")
            nc.sync.dma_start(
                out=adj3_sb[:],
                in_=adjT[g].rearrange("(s p) t -> p s t", p=128),
            )
            adj_sb = [adj3_sb[: k1 - k0, ki, :] for ki, (k0, k1) in enumerate(KD)]
            adjF8_sb = gpool3.tile([128, 2, TOK], F8, tag="adjF8", name=f"adjF8_{g}")
            nc.sync.dma_start(out=adjF8_sb[:], in_=adjF8_d[g, :, :, :])
            on3_sb = spool.tile([128, 3, N], BF, tag="on3", name=f"on3_{g}")
            nc.sync.dma_start(
                out=on3_sb[:],
                in_=onT[g].rearrange("(s p) n -> p s n", p=128),
            )
            on_sb = [on3_sb[: k1 - k0, ki, :] for ki, (k0, k1) in enumerate(KD)]
            st["adj"] = adj_sb
            st["adjF8"] = adjF8_sb

            # q = Wq @ ques + bq
            q_ps = ps1.tile([128, 4], F32, tag="ps1", name=f"qps_{g}")
            for m, (m0, m1) in enumerate(MS):
                for k in range(4):
                    nc.tensor.matmul(
                        q_ps[:, m : m + 1],
                        wq_sb[k][:, m0:m1],
                        ques_sb[:, k : k + 1],
                        start=(k == 0),
                        stop=(k == 3),
                    )
            q_sb = spool.tile([128, 4], F32, tag="q", name=f"q_{g}")
            for m in range(4):
                nc.vector.tensor_scalar_add(
                    q_sb[:, m : m + 1], q_ps[:, m : m + 1], bq_sb[:, m : m + 1]
                )

            # fold q into We, Wv (host pre-scaled x32) -> fp8 DR layout.
            # For the first two groups the scalar engine is the startup
            # critical path (ontT/w1x chain), so fold on the idle DVE there.
            weq_sb = gpool3.tile([128, 4, H], F8, tag="weq", name=f"weq_{g}")
            wvq_sb = gpool3.tile([128, 4, H], F8, tag="wvq", name=f"wvq_{g}")
            for dst, wsrc in ((weq_sb, we_sb), (wvq_sb, wv_sb)):
                for k in range(4):
                    if g < 2:
                        nc.vector.tensor_scalar_mul(
                            dst[:, k, :], wsrc[k][:], q_sb[:, k : k + 1]
                        )
                    else:
                        nc.scalar.activation(
                            out=dst[:, k, :], in_=wsrc[k][:], func=ACTF.Copy,
                            scale=q_sb[:, k : k + 1],
                        )
            st["weq"] = weq_sb
            st["wvq"] = wvq_sb

            # transposed on-term: ontT[n, c] = sum_f on[f, n] * W1a[f, c]
            # (lhsT = on chunk, rhs = W1a chunk -- no transpose needed).
            # Packed stage-A operands: one K=125 matmul covers
            # [ontT (80) | b1 (1) | w1b chunk3 (44)] against
            # [smat (80) | ones (1) | adj chunk3 (44)], folding the
            # on-term + bias + ragged D-chunk into a single accumulation.
            # All three row groups carry the x32 fp8 scale.
            ontT_ps = ps1.tile([N, H], F32, tag="ps1", name=f"ontTps_{g}")
            for ki in range(3):
                nc.tensor.matmul(
                    ontT_ps[:],
                    on_sb[ki][:],
                    w1a_sb[ki][:],
                    start=(ki == 0),
                    stop=(ki == 2),
                )
            KX = N + 1 + (D - 256)  # 125
            w1x_sb = spool.tile([KX, H], BF, tag="w1x", name=f"w1x_{g}")
            nc.scalar.activation(
                out=w1x_sb[:N, :], in_=ontT_ps[:], func=ACTF.Copy, scale=SC
            )
            nc.sync.dma_start(out=w1x_sb[N : N + 1, :], in_=b1row_d[:, :])
            nc.sync.dma_start(out=w1x_sb[N + 1 :, :], in_=w1b44_d[:, :])
            adjx_sb = gpool3.tile([KX, TOK], BF, tag="adjx", name=f"adjx_{g}")
            nc.sync.dma_start(out=adjx_sb[: N + 1, :], in_=smat_d[:, :])
            nc.sync.dma_start(out=adjx_sb[N + 1 :, :], in_=adjT[g, 256:D, :])
            st["w1x"] = w1x_sb
            st["adjx"] = adjx_sb
            return st

        def ps_pair(name):
            pa = ps2.tile([128, 2, 512], F32, tag="ps2", name=f"{name}a")
            pb = ps2.tile([128, 2, 512], F32, tag="ps2", name=f"{name}b")
            return pa, pb

        def pst(pa, pb, t):
            return (pa if t < 2 else pb)[:, t % 2, 0:T]

        def emit_AB(g, st):
            """Stage A/B + softmax-a chain (C/D)."""
            adj_sb = st["adj"]
            adjF8_sb = st["adjF8"]
            w1x_sb = st["w1x"]
            adjx_sb = st["adjx"]
            weq_sb = st["weq"]

            # edges (x32, fp8, [128, mchunk, tok]); expa = exp(logits_a) bf16
            edges_sb = gpool.tile([128, 4, TOK], F8, tag="edges", name=f"edges_{g}")
            expa_sb = gpool.tile([128, 4, TOK], BF, tag="expa", name=f"expa_{g}")
            st["edges"] = edges_sb
            st["expa"] = expa_sb

            # stage A: 32*edges = (32 W1b[0:256]) @ adj  (fp8 DoubleRow)
            #        + [32 ontT | 32 b1 | 32 W1b44] @ [smat | ones | adj44]
            for m, (m0, m1) in enumerate(MS):
                pa, pb = ps_pair(f"eps_{g}_{m}")
                # snake the two weight groups so adjacent m-blocks share a
                # boundary stationary operand (dedupe drops the reload)
                parts = [
                    lambda s, e: [
                        nc.tensor.matmul(
                            pst(pa, pb, t), w1bDR_sb[:, :, m0:m1],
                            adjF8_sb[:, :, tsl(t)],
                            start=s, stop=e, perf_mode=DR,
                        )
                        for t in range(NT)
                    ],
                    lambda s, e: [
                        nc.tensor.matmul(
                            pst(pa, pb, t), w1x_sb[:, m0:m1],
                            adjx_sb[:, tsl(t)], start=s, stop=e,
                        )
                        for t in range(NT)
                    ],
                ]
                if m % 2:
                    parts.reverse()
                parts[0](True, False)
                parts[1](False, True)
                ev = _re4(edges_sb[:, m, :])
                nc.scalar.copy(out=ev[:, 0:2, :], in_=pa[:, :, 0:T])
                nc.scalar.copy(out=ev[:, 2:4, :], in_=pb[:, :, 0:T])

            # stage B: expa = exp((1/1024) * (32 We q) @ (32 edges) + be)
            e3 = edges_sb
            for m, (m0, m1) in enumerate(MS):
                pa, pb = ps_pair(f"lps_{g}_{m}")
                order = (0, 1) if m % 2 == 0 else (1, 0)
                for j, i in enumerate(order):
                    for t in range(NT):
                        nc.tensor.matmul(
                            pst(pa, pb, t),
                            weq_sb[:, 2 * i : 2 * i + 2, m0:m1],
                            e3[:, 2 * i : 2 * i + 2, tsl(t)],
                            start=(j == 0),
                            stop=(j == 1),
                            perf_mode=DR,
                        )
                ea = _re4(expa_sb[:, m, :])
                nc.scalar.activation(
                    out=ea[:, 0:2, :], in_=pa[:, :, 0:T], func=ACTF.Exp,
                    bias=be_sb[:, m : m + 1], scale=ISC2,
                )
                nc.scalar.activation(
                    out=ea[:, 2:4, :], in_=pb[:, :, 0:T], func=ACTF.Exp,
                    bias=be_sb[:, m : m + 1], scale=ISC2,
                )

            # stage C: suma, reca = 1/suma (fast-approx NR reciprocal)
            suma = spool.tile([128, 4, N], F32, tag="suma", name=f"suma_{g}")
            for m in range(4):
                nc.vector.tensor_reduce(
                    suma[:, m, :], _re3(expa_sb[:, m, :]), axis=AX.X, op=ALU.add
                )
            reca = spool.tile([128, 4, N], F32, tag="reca", name=f"reca_{g}")
            nc.vector.reciprocal(reca[:], suma[:])

            # stage D: a = expa * reca (in place, gpsimd);
            # edges2 = a * (32 edges) -> fp8 (gpsimd; DVE pays a ~2x
            # penalty on fp8-operand tensor_tensor)
            edges2_sb = gpool.tile([128, 4, TOK], F8, tag="e2", name=f"e2_{g}")
            st["edges2"] = edges2_sb
            for m in range(4):
                eam = _re3(expa_sb[:, m, :])
                rb = reca[:, m, :]
                nc.gpsimd.tensor_tensor(
                    out=eam,
                    in0=eam,
                    in1=rb[:, :, None].broadcast_to((128, N, E)),
                    op=ALU.mult,
                )
                e2eng = nc.vector if (g == 0 and m >= 2) else nc.gpsimd
                e2eng.tensor_tensor(
                    out=edges2_sb[:, m, :],
                    in0=expa_sb[:, m, :],
                    in1=edges_sb[:, m, :],
                    op=ALU.mult,
                )
            return st

        def emit_EF(g, st):
            """Stages E and F for group g."""
            adj_sb = st["adj"]
            adjF8_sb = st["adjF8"]
            edges2_sb = st["edges2"]
            wvq_sb = st["wvq"]

            t_sb = gpool.tile([128, 4, TOK], F8, tag="tt", name=f"t_{g}")
            expb_sb = gpool.tile([128, 4, TOK], BF, tag="expb", name=f"expb_{g}")
            st["expb"] = expb_sb

            # stage E: 32*t = (32 W2a[0:256]) @ adj (DR) + (32 W2a44) @ adj44
            #        + W2b @ (32 edges2) (DR); +32*b2 at the drain
            for m, (m0, m1) in enumerate(MS):
                pa, pb = ps_pair(f"tps_{g}_{m}")
                def mk_dr(wslice, rslice_t):
                    def emit(s, e):
                        for t in range(NT):
                            nc.tensor.matmul(
                                pst(pa, pb, t), wslice, rslice_t(t),
                                start=s, stop=e, perf_mode=DR,
                            )
                    return emit
                def mk_bf(wslice, rslice_t):
                    def emit(s, e):
                        for t in range(NT):
                            nc.tensor.matmul(
                                pst(pa, pb, t), wslice, rslice_t(t),
                                start=s, stop=e,
                            )
                    return emit
                parts = [
                    mk_dr(w2aDR_sb[:, :, m0:m1],
                          lambda t: adjF8_sb[:, :, tsl(t)]),
                    mk_dr(w2bDR_sb[:, 0:2, m0:m1],
                          lambda t: edges2_sb[:, 0:2, tsl(t)]),
                    mk_dr(w2bDR_sb[:, 2:4, m0:m1],
                          lambda t: edges2_sb[:, 2:4, tsl(t)]),
                    mk_bf(w2a44_sb[:, m0:m1],
                          lambda t: adj_sb[2][:, tsl(t)]),
                ]
                if m % 2:
                    parts.reverse()
                for j, p in enumerate(parts):
                    p(j == 0, j == len(parts) - 1)
                tv = _re4(t_sb[:, m, :])
                nc.scalar.activation(
                    out=tv[:, 0:2, :], in_=pa[:, :, 0:T], func=ACTF.Identity,
                    bias=b2s_sb[:, m : m + 1],
                )
                nc.scalar.activation(
                    out=tv[:, 2:4, :], in_=pb[:, :, 0:T], func=ACTF.Identity,
                    bias=b2s_sb[:, m : m + 1],
                )

            # stage F: expb = exp((1/1024) * (32 Wv q) @ (32 t) + bv)
            for m, (m0, m1) in enumerate(MS):
                pa, pb = ps_pair(f"bps_{g}_{m}")
                order = (0, 1) if m % 2 == 0 else (1, 0)
                for j, i in enumerate(order):
                    for t in range(NT):
                        nc.tensor.matmul(
                            pst(pa, pb, t),
                            wvq_sb[:, 2 * i : 2 * i + 2, m0:m1],
                            t_sb[:, 2 * i : 2 * i + 2, tsl(t)],
                            start=(j == 0),
                            stop=(j == 1),
                            perf_mode=DR,
                        )
                eb = _re4(expb_sb[:, m, :])
                nc.scalar.activation(
                    out=eb[:, 0:2, :], in_=pa[:, :, 0:T], func=ACTF.Exp,
                    bias=bv_sb[:, m : m + 1], scale=ISC2,
                )
                nc.scalar.activation(
                    out=eb[:, 2:4, :], in_=pb[:, :, 0:T], func=ACTF.Exp,
                    bias=bv_sb[:, m : m + 1], scale=ISC2,
                )

        def emit_GHI(g, st):
            """Stages G..I for group g."""
            adj_sb = st["adj"]
            expb_sb = st["expb"]

            # stage G: sumb, recb
            sumb = spool.tile([128, 4, N], F32, tag="sumb", name=f"sumb_{g}")
            for m in range(4):
                nc.vector.tensor_reduce(
                    sumb[:, m, :], _re3(expb_sb[:, m, :]), axis=AX.X, op=ALU.add
                )
            recb = spool.tile([128, 4, N], F32, tag="recb", name=f"recb_{g}")

            # stage H: pre = (Wadj @ adj + badj) * expb  (into expb_sb, bf16)
            for m, (m0, m1) in enumerate(MS):
                pa, pb = ps_pair(f"aps_{g}_{m}")
                korder = (0, 1, 2) if m % 2 == 0 else (2, 1, 0)
                for j, ki in enumerate(korder):
                    for t in range(NT):
                        nc.tensor.matmul(
                            pst(pa, pb, t),
                            wadj_sb[ki][:, m0:m1],
                            adj_sb[ki][:, tsl(t)],
                            start=(j == 0),
                            stop=(j == 2),
                        )
                eb = _re4(expb_sb[:, m, :])
                nc.vector.scalar_tensor_tensor(
                    out=eb[:, 0:2, :],
                    in0=pa[:, :, 0:T],
                    scalar=badj_sb[:, m : m + 1],
                    in1=eb[:, 0:2, :],
                    op0=ALU.add,
                    op1=ALU.mult,
                )
                nc.vector.scalar_tensor_tensor(
                    out=eb[:, 2:4, :],
                    in0=pb[:, :, 0:T],
                    scalar=badj_sb[:, m : m + 1],
                    in1=eb[:, 2:4, :],
                    op0=ALU.add,
                    op1=ALU.mult,
                )

            # stage I: out = (sum_E pre) * recb ; store
            s_sb = spool.tile([128, 4, N], F32, tag="s", name=f"s_{g}")
            for m in range(4):
                nc.vector.tensor_reduce(
                    s_sb[:, m, :], _re3(expb_sb[:, m, :]), axis=AX.X, op=ALU.add
                )
            # reciprocal sits after the PSUM-freeing drains on the DVE queue
            nc.vector.reciprocal(recb[:], sumb[:])
            o_sb = spool.tile([128, 4, N], F32, tag="o", name=f"o_{g}")
            nc.gpsimd.tensor_tensor(
                out=o_sb[:], in0=s_sb[:], in1=recb[:], op=ALU.mult
            )
            nc.sync.dma_start(
                out=outT[g].rearrange("f p n -> p f n"), in_=o_sb[:]
            )

        # software pipeline across groups, 3 stages deep:
        #   ... AB(g) | EF(g-1) | preamble(g+1) | GHI(g-1) ...
        # The serial preamble chain (q -> weight folds -> on-term expand) of
        # g+1 is tucked after stage F's ACT work so it never sits between
        # stage-critical ACT/DVE ops, and completes long before AB(g+1).
        states = {0: emit_preamble(0), 1: emit_preamble(1)}
        for g in range(G):
            emit_AB(g, states[g])
            if g >= 1:
                emit_EF(g - 1, states[g - 1])
            if g >= 1 and g + 1 < G:
                states[g + 1] = emit_preamble(g + 1)
            if g >= 1:
                emit_GHI(g - 1, states.pop(g - 1))
        emit_EF(G - 1, states[G - 1])
        emit_GHI(G - 1, states.pop(G - 1))

    nsplit = _split_multi_waits(nc)
    if os.environ.get("KERNEL_DEBUG"):
        print(f"split_multi_waits: {nsplit} nops inserted", file=sys.stderr)
    return nc


def _pack_bias(b):
    # [H] -> [128, 4]: column j = channels j*128..(j+1)*128
    return np.ascontiguousarray(np.asarray(b, np.float32).reshape(4, 128).T)


def _bf(x):
    return np.ascontiguousarray(np.asarray(x, np.float32).astype(ml_dtypes.bfloat16))


def _f8(x):
    x = np.clip(np.asarray(x, np.float32), -240.0, 240.0)
    return np.ascontiguousarray(x.astype(ml_dtypes.float8_e4m3))


def _smat():
    """[N+1, TOK] node->token selection matrix (+ ones row for the b1 bias)."""
    s = np.zeros((N + 1, TOK), np.float32)
    for n in range(N):
        s[n, n * E : (n + 1) * E] = 1.0
    s[N, :] = 1.0
    return _bf(s)


def prepare_inputs(ques_embed, adj_list, original_nodes,
                   w1_w, w1_b, wq_w, wq_b, we_w, we_b,
                   w2_w, w2_b, wv_w, wv_b, wadj_w, wadj_b):
    """Host-side layout prep: feature-major tensors, fp8 DoubleRow weight
    layouts (x32), plus per-core shards. Returns per-core input maps."""
    adjTf = np.asarray(adj_list, np.float32).reshape(BR, TOK, D).transpose(0, 2, 1)
    adjT = _bf(np.concatenate(
        [adjTf, np.zeros((BR, 384 - D, TOK), np.float32)], axis=1))
    # fp8 copy of adj rows 0..255, DoubleRow layout [128, 2, TOK]
    adjF8 = _f8(adjTf[:, :256, :].reshape(BR, 2, 128, TOK).transpose(0, 2, 1, 3))
    onTf = np.asarray(original_nodes, np.float32).reshape(BR, N, D).transpose(0, 2, 1)
    onT = _bf(np.concatenate(
        [onTf, np.zeros((BR, 384 - D, N), np.float32)], axis=1))
    quesT = _bf(
        np.asarray(ques_embed, np.float32).reshape(BR, 4, 128).transpose(0, 2, 1)
    )

    w1b = np.asarray(w1_w, np.float32)[:, D:].T    # [D, H]
    w2a = np.asarray(w2_w, np.float32)[:, :D].T    # [D, H]
    w2b = np.asarray(w2_w, np.float32)[:, D:].T    # [H, H]

    def _dr2(w):
        # [256, H] -> [128, 2, H] DoubleRow pairing (slot i = rows 128i..)
        return w[:256].reshape(2, 128, H).transpose(1, 0, 2)

    def _chunk3(wdh):
        # [D, H] -> [128, 3, H] (slot s = rows 128s.., zero-padded)
        p = np.concatenate([np.asarray(wdh, np.float32),
                            np.zeros((384 - D, H), np.float32)], axis=0)
        return p.reshape(3, 128, H).transpose(1, 0, 2)

    def _chunk4(whh):
        return np.asarray(whh, np.float32).reshape(4, 128, H).transpose(1, 0, 2)

    w = {
        "w1a": _bf(_chunk3(np.asarray(w1_w)[:, :D].T)),
        "w1bDR": _f8(SC * _dr2(w1b)),
        "w1b44": _bf(SC * w1b[256:D]),
        "w2aDR": _f8(SC * _dr2(w2a)),
        "w2a44": _bf(SC * w2a[256:D]),
        "w2bDR": _f8(w2b.reshape(4, 128, H).transpose(1, 0, 2)),
        "wadj": _bf(_chunk3(np.asarray(wadj_w).T)),
        "wq": _bf(_chunk4(np.asarray(wq_w).T)),
        "we": _bf(_chunk4(SC * np.asarray(we_w, np.float32).T)),
        "wv": _bf(_chunk4(SC * np.asarray(wv_w, np.float32).T)),
        "bias": np.ascontiguousarray(np.concatenate([
            _pack_bias(wq_b), _pack_bias(we_b),
            _pack_bias(SC * np.asarray(w2_b, np.float32)),
            _pack_bias(wv_b), _pack_bias(wadj_b)], axis=1)),
        "b1row": _bf(SC * np.asarray(w1_b, np.float32).reshape(1, H)),
        "smat": _smat(),
    }

    in_maps = []
    for c in range(NCORES):
        sl = slice(c * G, (c + 1) * G)
        m = dict(w)
        m["adjT"] = np.ascontiguousarray(adjT[sl])
        m["adjF8"] = np.ascontiguousarray(adjF8[sl])
        m["onT"] = np.ascontiguousarray(onT[sl])
        m["quesT"] = np.ascontiguousarray(quesT[sl])
        in_maps.append(m)
    return in_maps


def run(in_maps, trace=False, tmpdir=None):
    _install_ntff_hook()
    if not os.environ.get("KERNEL_NO_LDW_DEDUPE"):
        _patch_ldw_dedupe()
    from concourse.bass_utils import run_bass_kernel_spmd

    nc = build_program()
    res = run_bass_kernel_spmd(
        nc,
        in_maps,
        core_ids=list(range(NCORES)),
        trace=trace,
        tmpdir=tmpdir,
    )
    return res


def gather_output(res):
    outT = np.stack([res.results[c]["outT"] for c in range(NCORES)])  # [8,5,4,128,N]
    outT = outT.reshape(BR, 4, 128, N).transpose(0, 3, 1, 2)          # [40,N,4,128]
    return np.ascontiguousarray(outT.reshape(B, R, N, H).astype(np.float32))


def kernel(ques_embed, adj_list, original_nodes,
           w1_w, w1_b, wq_w, wq_b, we_w, we_b,
           w2_w, w2_b, wv_w, wv_b, wadj_w, wadj_b,
           deg=None, batch_size=None, **_unused):
    in_maps = prepare_inputs(
        ques_embed, adj_list, original_nodes,
        w1_w, w1_b, wq_w, wq_b, we_w, we_b,
        w2_w, w2_b, wv_w, wv_b, wadj_w, wadj_b,
    )
    res = run(in_maps, trace=False)
    return gather_output(res)
